# revision 18
# baseline (speedup 1.0000x reference)
"""Trainium2 Bass kernel (raw Bass, no Tile): per-class precision/recall sums.

Computes, for pred/gt 0-1 indicator tensors of shape [N, C]:
    intersection = sum_n pred*gt   [C]
    pred_sum     = sum_n pred      [C]
    gt_sum       = sum_n gt        [C]
    precisions   = (intersection + EPS) / (pred_sum + EPS)
    recalls      = (intersection + EPS) / (gt_sum + EPS)

Sharding: rows split across 8 NeuronCores. The host interleaves each
core's pred/gt chunks in 8-row blocks into x[R/8, 2, 8, C] so one DMA
per tile feeds both operands (each 128-element run purely pred or gt).
Each core emits a [1, 3*C] partial; the host sums partials (integer
values < 2^24, exact in fp32) and applies the epsilon math.

Device pipeline per core (memory-bound, 64 MiB HBM traffic):
  - gpsimd SWDGE DMAs cast f32 -> bf16 in flight (exact for 0/1):
    32 tiles xt[128, 4096] bf16 into 16 rotating SBUF slots.
  - TensorE does all the math:
    * ones[128,1]^T @ 512-col slices accumulate ps_sums[1,512].
    * Gram matmuls pred_run^T @ gt_run per 256-col block accumulate
      ps_gram[128,128]; diagonal entry a=(r,c) = pred.gt dot.
  - Epilogue: diag-mask ps_gram (affine_select identity), fp32
    ones-matmul column-sum -> ps_row[1,128], strided reduces fold into
    res[1,48] = [pred_sum, gt_sum, intersection].

Raw Bass because this compiler build encodes at most ONE semaphore wait
per TPB instruction: all multi-condition waits are standalone sequencer
wait_ge instructions. Correctness argument for slot recycling: the DMA
for tile t (t>=16) waits pe_sem >= t-15, i.e. PE finished reading tile
t-16 in that slot, which also implies that DMA t-16 completed.
Per-slot DMA-completion sems make PE's data waits exact even if the
runtime spreads DMAs across queues that complete out of order.
"""

from contextlib import ExitStack

import numpy as np

N_CORES = 8
N_ROWS, C = 4194304, 16
ROWS_PER_CORE = N_ROWS // N_CORES  # 524288
EPS = np.float32(1e-6)

P = 128
ELEMS_PER_CORE = ROWS_PER_CORE * 2 * C      # 16777216
# Graded tiles: big tiles stream at max efficiency; small tiles at the
# end shrink the post-stream PE chase to ~2us.
FREE_B = 8192        # bf16 elements per partition per big tile
N_BIG = 14
N_BIG_SLOTS = 9
FREE_S = 2048
N_SMALL = 8          # 14*8192 + 8*2048 = 131072 = ELEMS/P
MM_FREE = 512
GRAM_BLK = 256       # (two=2, r=8, c=16)
N_TILES = N_BIG + N_SMALL

_CACHE = {}
LAST_RUN = None  # BassKernelResults of the most recent run (for test harness)


def _build_nc():
    import concourse.bass as bass
    import concourse.mybir as mybir

    f32 = mybir.dt.float32
    bf16 = mybir.dt.bfloat16

    nc = bass.Bass()
    x_d = nc.dram_tensor("x", [ROWS_PER_CORE // 8, 2, 8, C], f32,
                         kind="ExternalInput")
    out_d = nc.dram_tensor("out", [1, 3 * C], f32, kind="ExternalOutput")

    rb_big = N_BIG * P * FREE_B // GRAM_BLK     # row-blocks in big region
    x_big = x_d[0:rb_big].rearrange("(t p f) two r c -> t p (f two r c)",
                                    p=P, f=FREE_B // GRAM_BLK)
    x_small = x_d[rb_big:].rearrange("(t p f) two r c -> t p (f two r c)",
                                     p=P, f=FREE_S // GRAM_BLK)

    ctx = ExitStack()
    with ctx:
        ones_b = ctx.enter_context(nc.sbuf_tensor("ones_b", [P, 1], bf16))
        ones_f = ctx.enter_context(nc.sbuf_tensor("ones_f", [P, 1], f32))
        onesI = ctx.enter_context(nc.sbuf_tensor("onesI", [P, P], f32))
        ident = ctx.enter_context(nc.sbuf_tensor("ident", [P, P], f32))
        diag = ctx.enter_context(nc.sbuf_tensor("diag", [P, P], f32))
        sum4 = ctx.enter_context(nc.sbuf_tensor("sum4", [1, 4 * C], f32))
        res = ctx.enter_context(nc.sbuf_tensor("res", [1, 3 * C], f32))
        slots = [
            ctx.enter_context(nc.sbuf_tensor(f"xt{s}", [P, FREE_B], bf16))
            for s in range(N_BIG_SLOTS)
        ]
        sslots = [
            ctx.enter_context(nc.sbuf_tensor(f"xs{s}", [P, FREE_S], bf16))
            for s in range(N_SMALL)
        ]

        ps_sums = ctx.enter_context(nc.psum_tensor([1, MM_FREE], f32))
        ps_gram = ctx.enter_context(nc.psum_tensor([P, P], f32))
        ps_row = ctx.enter_context(nc.psum_tensor([1, P], f32))

        slot_sems = [
            ctx.enter_context(nc.semaphore(name=f"slot{s}"))
            for s in range(N_BIG_SLOTS)
        ]
        sslot_sems = [
            ctx.enter_context(nc.semaphore(name=f"sslot{s}"))
            for s in range(N_SMALL)
        ]
        pe_sem = ctx.enter_context(nc.semaphore(name="pe"))
        dve_sem = ctx.enter_context(nc.semaphore(name="dve"))
        pool_sem = ctx.enter_context(nc.semaphore(name="pool"))
        out_sem = ctx.enter_context(nc.semaphore(name="outd"))
        block = ctx.enter_context(nc.Block())

        @block.gpsimd
        def _(gpsimd):
            gpsimd.memset(onesI[:], 1.0)
            gpsimd.affine_select(ident[:], onesI[:], [[1, P]],
                                 mybir.AluOpType.is_equal, 0.0,
                                 base=0, channel_multiplier=-1)
            gpsimd.nop().then_inc(pool_sem, 1)
            for t in range(N_BIG):
                s = t % N_BIG_SLOTS
                if t >= N_BIG_SLOTS:
                    # PE finished reading the previous occupant of this slot
                    gpsimd.wait_ge(pe_sem, t - N_BIG_SLOTS + 1)
                gpsimd.dma_start(slots[s][:], x_big[t]).then_inc(
                    slot_sems[s], 16)
            for u in range(N_SMALL):
                gpsimd.dma_start(sslots[u][:], x_small[u]).then_inc(
                    sslot_sems[u], 16)
            # final output DMA after DVE finishes the epilogue
            gpsimd.wait_ge(dve_sem, 3)
            gpsimd.dma_start(out_d[:, :], res[:]).then_inc(out_sem, 16)
            gpsimd.wait_ge(out_sem, 16)

        @block.vector
        def _(vector):
            vector.memset(ones_b[:], 1.0)
            vector.memset(ones_f[:], 1.0)
            vector.nop().then_inc(dve_sem, 1)
            # epilogue part 1: after all accumulation matmuls
            vector.wait_ge(pe_sem, N_TILES)
            vector.wait_ge(pool_sem, 1)
            vector.tensor_mul(diag[:], ps_gram[:, :], ident[:])
            vector.tensor_reduce(
                sum4[:],
                ps_sums[:, :].rearrange("p (b2 two r c) -> p b2 two c r",
                                        b2=2, two=2, r=8, c=C),
                axis=mybir.AxisListType.X, op=mybir.AluOpType.add)
            vector.tensor_reduce(
                res[0:1, 0:2 * C],
                sum4[:, :].rearrange("p (b2 tc) -> p tc b2", b2=2, tc=2 * C),
                axis=mybir.AxisListType.X, op=mybir.AluOpType.add)
            vector.nop().then_inc(dve_sem, 1)  # diag + sums folded
            # epilogue part 2: after PE's diag column-sum matmul
            vector.wait_ge(pe_sem, N_TILES + 1)
            vector.tensor_reduce(
                res[0:1, 2 * C:3 * C],
                ps_row[:, :].rearrange("p (g c) -> p c g", g=8, c=C),
                axis=mybir.AxisListType.X, op=mybir.AluOpType.add)
            vector.nop().then_inc(dve_sem, 1)

        @block.tensor
        def _(tensor):
            tensor.wait_ge(dve_sem, 1)  # ones_b / ones_f ready
            n_sum_mms = (N_BIG * FREE_B + N_SMALL * FREE_S) // MM_FREE
            n_gram_mms = (N_BIG * FREE_B + N_SMALL * FREE_S) // GRAM_BLK
            sum_mm = 0
            gram_mm = 0
            for t in range(N_TILES):
                if t < N_BIG:
                    xt = slots[t % N_BIG_SLOTS]
                    tensor.wait_ge(slot_sems[t % N_BIG_SLOTS],
                                   16 * (t // N_BIG_SLOTS + 1))
                    free = FREE_B
                else:
                    xt = sslots[t - N_BIG]
                    tensor.wait_ge(sslot_sems[t - N_BIG], 16)
                    free = FREE_S
                for i in range(free // MM_FREE):
                    nc.tensor.matmul(
                        ps_sums[:, :], ones_b[:],
                        xt[:, i * MM_FREE:(i + 1) * MM_FREE],
                        start=(sum_mm == 0), stop=(sum_mm == n_sum_mms - 1))
                    sum_mm += 1
                for j in range(free // GRAM_BLK):
                    base = j * GRAM_BLK
                    mminst = nc.tensor.matmul(
                        ps_gram[:, :], xt[:, base:base + P],
                        xt[:, base + P:base + 2 * P],
                        start=(gram_mm == 0),
                        stop=(gram_mm == n_gram_mms - 1))
                    gram_mm += 1
                    if j == free // GRAM_BLK - 1:
                        mminst.then_inc(pe_sem, 1)
            # epilogue: fp32 column-sum of masked diagonal
            tensor.wait_ge(dve_sem, 2)
            nc.tensor.matmul(ps_row[:, :], ones_f[:], diag[:],
                             start=True, stop=True).then_inc(pe_sem, 1)

    return nc


def _get_nc():
    if "nc" not in _CACHE:
        _CACHE["nc"] = _build_nc()
    return _CACHE["nc"]


def kernel(pred, gt, **run_kwargs):
    global LAST_RUN
    from concourse.bass_utils import run_bass_kernel_spmd

    pred = np.asarray(pred, dtype=np.float32)
    gt = np.asarray(gt, dtype=np.float32)
    assert pred.shape == (N_ROWS, C) and gt.shape == (N_ROWS, C)

    in_maps = []
    for i in range(N_CORES):
        sl = slice(i * ROWS_PER_CORE, (i + 1) * ROWS_PER_CORE)
        x = np.empty((ROWS_PER_CORE // 8, 2, 8, C), dtype=np.float32)
        x[:, 0, :, :] = pred[sl].reshape(-1, 8, C)
        x[:, 1, :, :] = gt[sl].reshape(-1, 8, C)
        in_maps.append({"x": x})

    nc = _get_nc()
    br = run_bass_kernel_spmd(nc, in_maps, core_ids=list(range(N_CORES)),
                              **run_kwargs)
    LAST_RUN = br

    partials = np.stack([r["out"].reshape(3 * C) for r in br.results])
    totals = partials.astype(np.float64).sum(axis=0)  # exact integers
    pred_sum = totals[0:C].astype(np.float32)
    gt_sum = totals[C:2 * C].astype(np.float32)
    intersection = totals[2 * C:3 * C].astype(np.float32)

    recalls = (intersection + EPS) / (gt_sum + EPS)
    precisions = (intersection + EPS) / (pred_sum + EPS)
    return (precisions, recalls, intersection, gt_sum, pred_sum)


# revision 19
# speedup vs baseline: 1.1637x; 1.1637x over previous
"""Trainium2 Bass kernel (raw Bass, no Tile): per-class precision/recall sums.

Computes, for pred/gt 0-1 indicator tensors of shape [N, C]:
    intersection = sum_n pred*gt   [C]
    pred_sum     = sum_n pred      [C]
    gt_sum       = sum_n gt        [C]
    precisions   = (intersection + EPS) / (pred_sum + EPS)
    recalls      = (intersection + EPS) / (gt_sum + EPS)

Sharding: rows split across 8 NeuronCores. The host interleaves each
core's pred/gt chunks in 8-row blocks into x[R/8, 2, 8, C] so one DMA
per tile feeds both operands (each 128-element run purely pred or gt).
Each core emits a [1, 3*C] partial; the host sums partials (integer
values < 2^24, exact in fp32) and applies the epsilon math.

Device pipeline per core (memory-bound, 64 MiB HBM traffic):
  - gpsimd SWDGE DMAs cast f32 -> bf16 in flight (exact for 0/1):
    32 tiles xt[128, 4096] bf16 into 16 rotating SBUF slots.
  - TensorE does all the math:
    * ones[128,1]^T @ 512-col slices accumulate ps_sums[1,512].
    * Gram matmuls pred_run^T @ gt_run per 256-col block accumulate
      ps_gram[128,128]; diagonal entry a=(r,c) = pred.gt dot.
  - Epilogue: diag-mask ps_gram (affine_select identity), fp32
    ones-matmul column-sum -> ps_row[1,128], strided reduces fold into
    res[1,48] = [pred_sum, gt_sum, intersection].

Raw Bass because this compiler build encodes at most ONE semaphore wait
per TPB instruction: all multi-condition waits are standalone sequencer
wait_ge instructions. Correctness argument for slot recycling: the DMA
for tile t (t>=16) waits pe_sem >= t-15, i.e. PE finished reading tile
t-16 in that slot, which also implies that DMA t-16 completed.
Per-slot DMA-completion sems make PE's data waits exact even if the
runtime spreads DMAs across queues that complete out of order.
"""

from contextlib import ExitStack

import numpy as np

N_CORES = 8
N_ROWS, C = 4194304, 16
ROWS_PER_CORE = N_ROWS // N_CORES  # 524288
EPS = np.float32(1e-6)

P = 128
ELEMS_PER_CORE = ROWS_PER_CORE * 2 * C      # 16777216
FREE = 8192          # bf16 elements per partition per tile
TILE_ELEMS = P * FREE                       # 1048576
N_TILES = ELEMS_PER_CORE // TILE_ELEMS      # 16
N_SLOTS = 10
MM_FREE = 512
N_SUM_SLICES = FREE // MM_FREE              # 16
GRAM_BLK = 256       # (two=2, r=8, c=16)
N_GRAM_BLKS = FREE // GRAM_BLK              # 32

_CACHE = {}
LAST_RUN = None  # BassKernelResults of the most recent run (for test harness)


def _build_nc():
    import concourse.bass as bass
    import concourse.mybir as mybir

    f32 = mybir.dt.float32
    bf16 = mybir.dt.bfloat16

    nc = bass.Bass()
    x_d = nc.dram_tensor("x", [ROWS_PER_CORE // 8, 2, 8, C], f32,
                         kind="ExternalInput")
    out_d = nc.dram_tensor("out", [1, 3 * C], f32, kind="ExternalOutput")

    x_t = x_d[:, :, :, :].rearrange("(t p f) two r c -> t p (f two r c)",
                                    p=P, f=FREE // GRAM_BLK)

    ctx = ExitStack()
    with ctx:
        ones_b = ctx.enter_context(nc.sbuf_tensor("ones_b", [P, 1], bf16))
        ones_f = ctx.enter_context(nc.sbuf_tensor("ones_f", [P, 1], f32))
        onesI = ctx.enter_context(nc.sbuf_tensor("onesI", [P, P], f32))
        ident = ctx.enter_context(nc.sbuf_tensor("ident", [P, P], f32))
        diag = ctx.enter_context(nc.sbuf_tensor("diag", [P, P], f32))
        sum4 = ctx.enter_context(nc.sbuf_tensor("sum4", [1, 4 * C], f32))
        res = ctx.enter_context(nc.sbuf_tensor("res", [1, 3 * C], f32))
        slots = [
            ctx.enter_context(nc.sbuf_tensor(f"xt{s}", [P, FREE], bf16))
            for s in range(N_SLOTS)
        ]

        ps_sums = ctx.enter_context(nc.psum_tensor([1, MM_FREE], f32))
        ps_gram = ctx.enter_context(nc.psum_tensor([P, P], f32))
        ps_row = ctx.enter_context(nc.psum_tensor([1, P], f32))

        slot_sems = [
            ctx.enter_context(nc.semaphore(name=f"slot{s}"))
            for s in range(N_SLOTS)
        ]
        pe_sem = ctx.enter_context(nc.semaphore(name="pe"))
        dve_sem = ctx.enter_context(nc.semaphore(name="dve"))
        pool_sem = ctx.enter_context(nc.semaphore(name="pool"))
        out_sem = ctx.enter_context(nc.semaphore(name="outd"))
        block = ctx.enter_context(nc.Block())

        @block.gpsimd
        def _(gpsimd):
            gpsimd.memset(onesI[:], 1.0)
            gpsimd.affine_select(ident[:], onesI[:], [[1, P]],
                                 mybir.AluOpType.is_equal, 0.0,
                                 base=0, channel_multiplier=-1)
            gpsimd.nop().then_inc(pool_sem, 1)
            for t in range(N_TILES):
                s = t % N_SLOTS
                if t >= N_SLOTS:
                    # PE finished reading the previous occupant of this slot
                    gpsimd.wait_ge(pe_sem, t - N_SLOTS + 1)
                gpsimd.dma_start(slots[s][:], x_t[t]).then_inc(
                    slot_sems[s], 16)
            # final output DMA after DVE finishes the epilogue
            gpsimd.wait_ge(dve_sem, 3)
            gpsimd.dma_start(out_d[:, :], res[:]).then_inc(out_sem, 16)
            gpsimd.wait_ge(out_sem, 16)

        @block.vector
        def _(vector):
            vector.memset(ones_b[:], 1.0)
            vector.memset(ones_f[:], 1.0)
            vector.nop().then_inc(dve_sem, 1)
            # epilogue part 1: after all accumulation matmuls
            vector.wait_ge(pe_sem, N_TILES)
            vector.wait_ge(pool_sem, 1)
            vector.tensor_mul(diag[:], ps_gram[:, :], ident[:])
            vector.tensor_reduce(
                sum4[:],
                ps_sums[:, :].rearrange("p (b2 two r c) -> p b2 two c r",
                                        b2=2, two=2, r=8, c=C),
                axis=mybir.AxisListType.X, op=mybir.AluOpType.add)
            vector.tensor_reduce(
                res[0:1, 0:2 * C],
                sum4[:, :].rearrange("p (b2 tc) -> p tc b2", b2=2, tc=2 * C),
                axis=mybir.AxisListType.X, op=mybir.AluOpType.add)
            vector.nop().then_inc(dve_sem, 1)  # diag + sums folded
            # epilogue part 2: after PE's diag column-sum matmul
            vector.wait_ge(pe_sem, N_TILES + 1)
            vector.tensor_reduce(
                res[0:1, 2 * C:3 * C],
                ps_row[:, :].rearrange("p (g c) -> p c g", g=8, c=C),
                axis=mybir.AxisListType.X, op=mybir.AluOpType.add)
            vector.nop().then_inc(dve_sem, 1)

        @block.tensor
        def _(tensor):
            tensor.wait_ge(dve_sem, 1)  # ones_b / ones_f ready
            for t in range(N_TILES):
                s = t % N_SLOTS
                tensor.wait_ge(slot_sems[s], 16 * (t // N_SLOTS + 1))
                xt = slots[s]
                for i in range(N_SUM_SLICES):
                    mm = t * N_SUM_SLICES + i
                    nc.tensor.matmul(
                        ps_sums[:, :], ones_b[:],
                        xt[:, i * MM_FREE:(i + 1) * MM_FREE],
                        start=(mm == 0),
                        stop=(mm == N_TILES * N_SUM_SLICES - 1))
                for j in range(N_GRAM_BLKS):
                    mm = t * N_GRAM_BLKS + j
                    base = j * GRAM_BLK
                    mminst = nc.tensor.matmul(
                        ps_gram[:, :], xt[:, base:base + P],
                        xt[:, base + P:base + 2 * P],
                        start=(mm == 0),
                        stop=(mm == N_TILES * N_GRAM_BLKS - 1))
                    if j == N_GRAM_BLKS - 1:
                        mminst.then_inc(pe_sem, 1)
            # epilogue: fp32 column-sum of masked diagonal
            tensor.wait_ge(dve_sem, 2)
            nc.tensor.matmul(ps_row[:, :], ones_f[:], diag[:],
                             start=True, stop=True).then_inc(pe_sem, 1)

    return nc


def _get_nc():
    if "nc" not in _CACHE:
        _CACHE["nc"] = _build_nc()
    return _CACHE["nc"]


def kernel(pred, gt, **run_kwargs):
    global LAST_RUN
    from concourse.bass_utils import run_bass_kernel_spmd

    pred = np.asarray(pred, dtype=np.float32)
    gt = np.asarray(gt, dtype=np.float32)
    assert pred.shape == (N_ROWS, C) and gt.shape == (N_ROWS, C)

    in_maps = []
    for i in range(N_CORES):
        sl = slice(i * ROWS_PER_CORE, (i + 1) * ROWS_PER_CORE)
        x = np.empty((ROWS_PER_CORE // 8, 2, 8, C), dtype=np.float32)
        x[:, 0, :, :] = pred[sl].reshape(-1, 8, C)
        x[:, 1, :, :] = gt[sl].reshape(-1, 8, C)
        in_maps.append({"x": x})

    nc = _get_nc()
    br = run_bass_kernel_spmd(nc, in_maps, core_ids=list(range(N_CORES)),
                              **run_kwargs)
    LAST_RUN = br

    partials = np.stack([r["out"].reshape(3 * C) for r in br.results])
    totals = partials.astype(np.float64).sum(axis=0)  # exact integers
    pred_sum = totals[0:C].astype(np.float32)
    gt_sum = totals[C:2 * C].astype(np.float32)
    intersection = totals[2 * C:3 * C].astype(np.float32)

    recalls = (intersection + EPS) / (gt_sum + EPS)
    precisions = (intersection + EPS) / (pred_sum + EPS)
    return (precisions, recalls, intersection, gt_sum, pred_sum)


# revision 20
# speedup vs baseline: 1.1675x; 1.0033x over previous
"""Trainium2 Bass kernel (raw Bass, no Tile): per-class precision/recall sums.

Computes, for pred/gt 0-1 indicator tensors of shape [N, C]:
    intersection = sum_n pred*gt   [C]
    pred_sum     = sum_n pred      [C]
    gt_sum       = sum_n gt        [C]
    precisions   = (intersection + EPS) / (pred_sum + EPS)
    recalls      = (intersection + EPS) / (gt_sum + EPS)

Sharding: rows split across 8 NeuronCores. The host interleaves each
core's pred/gt chunks in 8-row blocks into x[R/8, 2, 8, C] so one DMA
per tile feeds both operands (each 128-element run purely pred or gt).
Each core emits a [1, 3*C] partial; the host sums partials (integer
values < 2^24, exact in fp32) and applies the epsilon math.

Device pipeline per core (memory-bound, 64 MiB HBM traffic):
  - gpsimd SWDGE DMAs cast f32 -> bf16 in flight (exact for 0/1):
    32 tiles xt[128, 4096] bf16 into 16 rotating SBUF slots.
  - TensorE does all the math:
    * ones[128,1]^T @ 512-col slices accumulate ps_sums[1,512].
    * Gram matmuls pred_run^T @ gt_run per 256-col block accumulate
      ps_gram[128,128]; diagonal entry a=(r,c) = pred.gt dot.
  - Epilogue: diag-mask ps_gram (affine_select identity), fp32
    ones-matmul column-sum -> ps_row[1,128], strided reduces fold into
    res[1,48] = [pred_sum, gt_sum, intersection].

Raw Bass because this compiler build encodes at most ONE semaphore wait
per TPB instruction: all multi-condition waits are standalone sequencer
wait_ge instructions. Correctness argument for slot recycling: the DMA
for tile t (t>=16) waits pe_sem >= t-15, i.e. PE finished reading tile
t-16 in that slot, which also implies that DMA t-16 completed.
Per-slot DMA-completion sems make PE's data waits exact even if the
runtime spreads DMAs across queues that complete out of order.
"""

from contextlib import ExitStack

import numpy as np

N_CORES = 8
N_ROWS, C = 4194304, 16
ROWS_PER_CORE = N_ROWS // N_CORES  # 524288
EPS = np.float32(1e-6)

P = 128
ELEMS_PER_CORE = ROWS_PER_CORE * 2 * C      # 16777216
FREE = 8192          # bf16 elements per partition per tile
TILE_ELEMS = P * FREE                       # 1048576
N_TILES = ELEMS_PER_CORE // TILE_ELEMS      # 16
N_SLOTS = 10
MM_FREE = 512
N_SUM_SLICES = FREE // MM_FREE              # 16
GRAM_BLK = 256       # (two=2, r=8, c=16)
N_GRAM_BLKS = FREE // GRAM_BLK              # 32

_CACHE = {}
LAST_RUN = None  # BassKernelResults of the most recent run (for test harness)


def _build_nc():
    import concourse.bass as bass
    import concourse.mybir as mybir

    f32 = mybir.dt.float32
    bf16 = mybir.dt.bfloat16

    nc = bass.Bass()
    x_d = nc.dram_tensor("x", [ROWS_PER_CORE // 8, 2, 8, C], f32,
                         kind="ExternalInput")
    out_d = nc.dram_tensor("out", [1, 3 * C], f32, kind="ExternalOutput")

    x_t = x_d[:, :, :, :].rearrange("(t p f) two r c -> t p (f two r c)",
                                    p=P, f=FREE // GRAM_BLK)

    ctx = ExitStack()
    with ctx:
        ones_b = ctx.enter_context(nc.sbuf_tensor("ones_b", [P, 1], bf16))
        ones_f = ctx.enter_context(nc.sbuf_tensor("ones_f", [P, 1], f32))
        onesI = ctx.enter_context(nc.sbuf_tensor("onesI", [P, P], f32))
        ident = ctx.enter_context(nc.sbuf_tensor("ident", [P, P], f32))
        diag = ctx.enter_context(nc.sbuf_tensor("diag", [P, P], f32))
        sum4 = ctx.enter_context(nc.sbuf_tensor("sum4", [1, 4 * C], f32))
        res = ctx.enter_context(nc.sbuf_tensor("res", [1, 3 * C], f32))
        slots = [
            ctx.enter_context(nc.sbuf_tensor(f"xt{s}", [P, FREE], bf16))
            for s in range(N_SLOTS)
        ]

        ps_sums = ctx.enter_context(nc.psum_tensor([1, MM_FREE], f32))
        ps_gram = ctx.enter_context(nc.psum_tensor([P, P], f32))
        ps_row = ctx.enter_context(nc.psum_tensor([1, P], f32))

        slot_sems = [
            ctx.enter_context(nc.semaphore(name=f"slot{s}"))
            for s in range(N_SLOTS)
        ]
        qsems = [
            ctx.enter_context(nc.semaphore(name=f"q{k}"))
            for k in range(4)
        ]
        pe_sem = ctx.enter_context(nc.semaphore(name="pe"))
        dve_sem = ctx.enter_context(nc.semaphore(name="dve"))
        pool_sem = ctx.enter_context(nc.semaphore(name="pool"))
        out_sem = ctx.enter_context(nc.semaphore(name="outd"))
        block = ctx.enter_context(nc.Block())

        @block.gpsimd
        def _(gpsimd):
            gpsimd.memset(onesI[:], 1.0)
            gpsimd.affine_select(ident[:], onesI[:], [[1, P]],
                                 mybir.AluOpType.is_equal, 0.0,
                                 base=0, channel_multiplier=-1)
            gpsimd.nop().then_inc(pool_sem, 1)
            for t in range(N_TILES):
                s = t % N_SLOTS
                if t >= N_SLOTS:
                    # PE finished reading the previous occupant of this slot
                    gpsimd.wait_ge(pe_sem, t - N_SLOTS + 1)
                if t < N_TILES - 1:
                    gpsimd.dma_start(slots[s][:], x_t[t]).then_inc(
                        slot_sems[s], 16)
                else:
                    # last tile: 4 quarter-DMAs so PE can chase the stream
                    # and finish right after the final byte lands
                    q = FREE // 4
                    for k in range(4):
                        gpsimd.dma_start(
                            slots[s][:, k * q:(k + 1) * q],
                            x_t[t][:, k * q:(k + 1) * q],
                        ).then_inc(qsems[k], 16)
            # final output DMA after DVE finishes the epilogue
            gpsimd.wait_ge(dve_sem, 3)
            gpsimd.dma_start(out_d[:, :], res[:]).then_inc(out_sem, 16)
            gpsimd.wait_ge(out_sem, 16)

        @block.vector
        def _(vector):
            vector.memset(ones_b[:], 1.0)
            vector.memset(ones_f[:], 1.0)
            vector.nop().then_inc(dve_sem, 1)
            # epilogue part 1: after all accumulation matmuls
            vector.wait_ge(pe_sem, N_TILES)
            vector.wait_ge(pool_sem, 1)
            vector.tensor_mul(diag[:], ps_gram[:, :], ident[:])
            vector.tensor_reduce(
                sum4[:],
                ps_sums[:, :].rearrange("p (b2 two r c) -> p b2 two c r",
                                        b2=2, two=2, r=8, c=C),
                axis=mybir.AxisListType.X, op=mybir.AluOpType.add)
            vector.tensor_reduce(
                res[0:1, 0:2 * C],
                sum4[:, :].rearrange("p (b2 tc) -> p tc b2", b2=2, tc=2 * C),
                axis=mybir.AxisListType.X, op=mybir.AluOpType.add)
            vector.nop().then_inc(dve_sem, 1)  # diag + sums folded
            # epilogue part 2: after PE's diag column-sum matmul
            vector.wait_ge(pe_sem, N_TILES + 1)
            vector.tensor_reduce(
                res[0:1, 2 * C:3 * C],
                ps_row[:, :].rearrange("p (g c) -> p c g", g=8, c=C),
                axis=mybir.AxisListType.X, op=mybir.AluOpType.add)
            vector.nop().then_inc(dve_sem, 1)

        @block.tensor
        def _(tensor):
            tensor.wait_ge(dve_sem, 1)  # ones_b / ones_f ready
            for t in range(N_TILES):
                s = t % N_SLOTS
                xt = slots[s]
                quarters = 1 if t < N_TILES - 1 else 4
                if quarters == 1:
                    tensor.wait_ge(slot_sems[s], 16 * (t // N_SLOTS + 1))
                for k in range(quarters):
                    if quarters == 4:
                        tensor.wait_ge(qsems[k], 16)
                    nsum = N_SUM_SLICES // quarters
                    ngram = N_GRAM_BLKS // quarters
                    for i in range(k * nsum, (k + 1) * nsum):
                        mm = t * N_SUM_SLICES + i
                        nc.tensor.matmul(
                            ps_sums[:, :], ones_b[:],
                            xt[:, i * MM_FREE:(i + 1) * MM_FREE],
                            start=(mm == 0),
                            stop=(mm == N_TILES * N_SUM_SLICES - 1))
                    for j in range(k * ngram, (k + 1) * ngram):
                        mm = t * N_GRAM_BLKS + j
                        base = j * GRAM_BLK
                        mminst = nc.tensor.matmul(
                            ps_gram[:, :], xt[:, base:base + P],
                            xt[:, base + P:base + 2 * P],
                            start=(mm == 0),
                            stop=(mm == N_TILES * N_GRAM_BLKS - 1))
                        if j == N_GRAM_BLKS - 1:
                            mminst.then_inc(pe_sem, 1)
            # epilogue: fp32 column-sum of masked diagonal
            tensor.wait_ge(dve_sem, 2)
            nc.tensor.matmul(ps_row[:, :], ones_f[:], diag[:],
                             start=True, stop=True).then_inc(pe_sem, 1)

    return nc


def _get_nc():
    if "nc" not in _CACHE:
        _CACHE["nc"] = _build_nc()
    return _CACHE["nc"]


def kernel(pred, gt, **run_kwargs):
    global LAST_RUN
    from concourse.bass_utils import run_bass_kernel_spmd

    pred = np.asarray(pred, dtype=np.float32)
    gt = np.asarray(gt, dtype=np.float32)
    assert pred.shape == (N_ROWS, C) and gt.shape == (N_ROWS, C)

    in_maps = []
    for i in range(N_CORES):
        sl = slice(i * ROWS_PER_CORE, (i + 1) * ROWS_PER_CORE)
        x = np.empty((ROWS_PER_CORE // 8, 2, 8, C), dtype=np.float32)
        x[:, 0, :, :] = pred[sl].reshape(-1, 8, C)
        x[:, 1, :, :] = gt[sl].reshape(-1, 8, C)
        in_maps.append({"x": x})

    nc = _get_nc()
    br = run_bass_kernel_spmd(nc, in_maps, core_ids=list(range(N_CORES)),
                              **run_kwargs)
    LAST_RUN = br

    partials = np.stack([r["out"].reshape(3 * C) for r in br.results])
    totals = partials.astype(np.float64).sum(axis=0)  # exact integers
    pred_sum = totals[0:C].astype(np.float32)
    gt_sum = totals[C:2 * C].astype(np.float32)
    intersection = totals[2 * C:3 * C].astype(np.float32)

    recalls = (intersection + EPS) / (gt_sum + EPS)
    precisions = (intersection + EPS) / (pred_sum + EPS)
    return (precisions, recalls, intersection, gt_sum, pred_sum)


# revision 22
# speedup vs baseline: 1.1705x; 1.0026x over previous
"""Trainium2 Bass kernel (raw Bass, no Tile): per-class precision/recall sums.

Computes, for pred/gt 0-1 indicator tensors of shape [N, C]:
    intersection = sum_n pred*gt   [C]
    pred_sum     = sum_n pred      [C]
    gt_sum       = sum_n gt        [C]
    precisions   = (intersection + EPS) / (pred_sum + EPS)
    recalls      = (intersection + EPS) / (gt_sum + EPS)

Sharding: rows split across 8 NeuronCores. The host interleaves each
core's pred/gt chunks in 8-row blocks into x[R/8, 2, 8, C] so one DMA
per tile feeds both operands (each 128-element run purely pred or gt).
Each core emits a [1, 3*C] partial; the host sums partials (integer
values < 2^24, exact in fp32) and applies the epsilon math.

Device pipeline per core (memory-bound, 64 MiB HBM traffic):
  - gpsimd SWDGE DMAs cast f32 -> bf16 in flight (exact for 0/1):
    32 tiles xt[128, 4096] bf16 into 16 rotating SBUF slots.
  - TensorE does all the math:
    * ones[128,1]^T @ 512-col slices accumulate ps_sums[1,512].
    * Gram matmuls pred_run^T @ gt_run per 256-col block accumulate
      ps_gram[128,128]; diagonal entry a=(r,c) = pred.gt dot.
  - Epilogue: diag-mask ps_gram (affine_select identity), fp32
    ones-matmul column-sum -> ps_row[1,128], strided reduces fold into
    res[1,48] = [pred_sum, gt_sum, intersection].

Raw Bass because this compiler build encodes at most ONE semaphore wait
per TPB instruction: all multi-condition waits are standalone sequencer
wait_ge instructions. Correctness argument for slot recycling: the DMA
for tile t (t>=16) waits pe_sem >= t-15, i.e. PE finished reading tile
t-16 in that slot, which also implies that DMA t-16 completed.
Per-slot DMA-completion sems make PE's data waits exact even if the
runtime spreads DMAs across queues that complete out of order.
"""

from contextlib import ExitStack

import numpy as np

N_CORES = 8
N_ROWS, C = 4194304, 16
ROWS_PER_CORE = N_ROWS // N_CORES  # 524288
EPS = np.float32(1e-6)

P = 128
ELEMS_PER_CORE = ROWS_PER_CORE * 2 * C      # 16777216
FREE = 8192          # bf16 elements per partition per tile
TILE_ELEMS = P * FREE                       # 1048576
N_TILES = ELEMS_PER_CORE // TILE_ELEMS      # 16
N_SLOTS = 10
MM_FREE = 512
N_SUM_SLICES = FREE // MM_FREE              # 16
GRAM_BLK = 256       # (two=2, r=8, c=16)
N_GRAM_BLKS = FREE // GRAM_BLK              # 32

_CACHE = {}
LAST_RUN = None  # BassKernelResults of the most recent run (for test harness)


def _build_nc():
    import concourse.bass as bass
    import concourse.mybir as mybir

    f32 = mybir.dt.float32
    bf16 = mybir.dt.bfloat16

    nc = bass.Bass()
    x_d = nc.dram_tensor("x", [ROWS_PER_CORE // 8, 2, 8, C], f32,
                         kind="ExternalInput")
    out_d = nc.dram_tensor("out", [1, 3 * C], f32, kind="ExternalOutput")

    x_t = x_d[:, :, :, :].rearrange("(t p f) two r c -> t p (f two r c)",
                                    p=P, f=FREE // GRAM_BLK)

    ctx = ExitStack()
    with ctx:
        ones_b = ctx.enter_context(nc.sbuf_tensor("ones_b", [P, 1], bf16))
        ones_f = ctx.enter_context(nc.sbuf_tensor("ones_f", [P, 1], f32))
        onesI = ctx.enter_context(nc.sbuf_tensor("onesI", [P, P], f32))
        ident = ctx.enter_context(nc.sbuf_tensor("ident", [P, P], f32))
        diag = ctx.enter_context(nc.sbuf_tensor("diag", [P, P], f32))
        sum4 = ctx.enter_context(nc.sbuf_tensor("sum4", [1, 4 * C], f32))
        res = ctx.enter_context(nc.sbuf_tensor("res", [1, 3 * C], f32))
        slots = [
            ctx.enter_context(nc.sbuf_tensor(f"xt{s}", [P, FREE], bf16))
            for s in range(N_SLOTS)
        ]

        ps_sums = ctx.enter_context(nc.psum_tensor([1, MM_FREE], f32))
        ps_gram = ctx.enter_context(nc.psum_tensor([P, P], f32))
        ps_row = ctx.enter_context(nc.psum_tensor([1, P], f32))

        slot_sems = [
            ctx.enter_context(nc.semaphore(name=f"slot{s}"))
            for s in range(N_SLOTS)
        ]
        qsems = [
            ctx.enter_context(nc.semaphore(name=f"q{k}"))
            for k in range(4)
        ]
        pe_sem = ctx.enter_context(nc.semaphore(name="pe"))
        dve_sem = ctx.enter_context(nc.semaphore(name="dve"))
        pool_sem = ctx.enter_context(nc.semaphore(name="pool"))
        out_sem = ctx.enter_context(nc.semaphore(name="outd"))
        block = ctx.enter_context(nc.Block())

        @block.gpsimd
        def _(gpsimd):
            gpsimd.memset(onesI[:], 1.0)
            gpsimd.affine_select(ident[:], onesI[:], [[1, P]],
                                 mybir.AluOpType.is_equal, 0.0,
                                 base=0, channel_multiplier=-1)
            gpsimd.nop().then_inc(pool_sem, 1)
            for t in range(N_TILES):
                s = t % N_SLOTS
                if t >= N_SLOTS:
                    # PE finished reading the previous occupant of this slot
                    gpsimd.wait_ge(pe_sem, t - N_SLOTS + 1)
                if t < N_TILES - 1:
                    gpsimd.dma_start(slots[s][:], x_t[t]).then_inc(
                        slot_sems[s], 16)
                else:
                    # last tile: 4 quarter-DMAs so PE can chase the stream
                    # and finish right after the final byte lands
                    q = FREE // 4
                    for k in range(4):
                        gpsimd.dma_start(
                            slots[s][:, k * q:(k + 1) * q],
                            x_t[t][:, k * q:(k + 1) * q],
                        ).then_inc(qsems[k], 16)
            # final output DMA after DVE finishes the epilogue
            gpsimd.wait_ge(dve_sem, 3)
            gpsimd.dma_start(out_d[:, :], res[:]).then_inc(out_sem, 16)
            gpsimd.wait_ge(out_sem, 16)

        @block.vector
        def _(vector):
            vector.memset(ones_b[:], 1.0)
            vector.memset(ones_f[:], 1.0)
            vector.nop().then_inc(dve_sem, 1)
            # epilogue part 1: after all accumulation matmuls
            vector.wait_ge(pe_sem, N_TILES)
            vector.wait_ge(pool_sem, 1)
            vector.tensor_mul(diag[:], ps_gram[:, :], ident[:])
            vector.tensor_reduce(
                sum4[:],
                ps_sums[:, :].rearrange("p (b2 two r c) -> p b2 two c r",
                                        b2=2, two=2, r=8, c=C),
                axis=mybir.AxisListType.X, op=mybir.AluOpType.add)
            vector.tensor_reduce(
                res[0:1, 0:2 * C],
                sum4[:, :].rearrange("p (b2 tc) -> p tc b2", b2=2, tc=2 * C),
                axis=mybir.AxisListType.X, op=mybir.AluOpType.add)
            vector.nop().then_inc(dve_sem, 1)  # diag + sums folded
            # epilogue part 2: after PE's diag column-sum matmul
            vector.wait_ge(pe_sem, N_TILES + 1)
            vector.tensor_reduce(
                res[0:1, 2 * C:3 * C],
                ps_row[:, :].rearrange("p (g c) -> p c g", g=8, c=C),
                axis=mybir.AxisListType.X, op=mybir.AluOpType.add)
            vector.nop().then_inc(dve_sem, 1)

        @block.tensor
        def _(tensor):
            tensor.wait_ge(dve_sem, 1)  # ones_b / ones_f ready
            for t in range(N_TILES):
                s = t % N_SLOTS
                xt = slots[s]
                quarters = 1 if t < N_TILES - 1 else 4
                if quarters == 1:
                    tensor.wait_ge(slot_sems[s], 16 * (t // N_SLOTS + 1))
                for k in range(quarters):
                    if quarters == 4:
                        tensor.wait_ge(qsems[k], 16)
                    nsum = N_SUM_SLICES // quarters
                    ngram = N_GRAM_BLKS // quarters
                    for i in range(k * nsum, (k + 1) * nsum):
                        mm = t * N_SUM_SLICES + i
                        nc.tensor.matmul(
                            ps_sums[:, :], ones_b[:],
                            xt[:, i * MM_FREE:(i + 1) * MM_FREE],
                            start=(mm == 0),
                            stop=(mm == N_TILES * N_SUM_SLICES - 1))
                    for j in range(k * ngram, (k + 1) * ngram):
                        mm = t * N_GRAM_BLKS + j
                        base = j * GRAM_BLK
                        mminst = nc.tensor.matmul(
                            ps_gram[:, :], xt[:, base:base + P],
                            xt[:, base + P:base + 2 * P],
                            start=(mm == 0),
                            stop=(mm == N_TILES * N_GRAM_BLKS - 1))
                        if j == N_GRAM_BLKS - 1:
                            mminst.then_inc(pe_sem, 1)
            # epilogue: fp32 column-sum of masked diagonal
            tensor.wait_ge(dve_sem, 2)
            nc.tensor.matmul(ps_row[:, :], ones_f[:], diag[:],
                             start=True, stop=True).then_inc(pe_sem, 1)

    return nc


def _get_nc():
    if "nc" not in _CACHE:
        _CACHE["nc"] = _build_nc()
    return _CACHE["nc"]


def kernel(pred, gt, **run_kwargs):
    global LAST_RUN
    from concourse.bass_utils import run_bass_kernel_spmd

    pred = np.asarray(pred, dtype=np.float32)
    gt = np.asarray(gt, dtype=np.float32)
    assert pred.shape == (N_ROWS, C) and gt.shape == (N_ROWS, C)

    in_maps = []
    for i in range(N_CORES):
        sl = slice(i * ROWS_PER_CORE, (i + 1) * ROWS_PER_CORE)
        x = np.empty((ROWS_PER_CORE // 8, 2, 8, C), dtype=np.float32)
        x[:, 0, :, :] = pred[sl].reshape(-1, 8, C)
        x[:, 1, :, :] = gt[sl].reshape(-1, 8, C)
        in_maps.append({"x": x})

    nc = _get_nc()
    br = run_bass_kernel_spmd(nc, in_maps, core_ids=list(range(N_CORES)),
                              **run_kwargs)
    LAST_RUN = br

    partials = np.stack([r["out"].reshape(3 * C) for r in br.results])
    totals = partials.astype(np.float64).sum(axis=0)  # exact integers
    pred_sum = totals[0:C].astype(np.float32)
    gt_sum = totals[C:2 * C].astype(np.float32)
    intersection = totals[2 * C:3 * C].astype(np.float32)

    recalls = (intersection + EPS) / (gt_sum + EPS)
    precisions = (intersection + EPS) / (pred_sum + EPS)
    return (precisions, recalls, intersection, gt_sum, pred_sum)
